# revision 9
# baseline (speedup 1.0000x reference)
"""Trainium2 Bass kernel for AttentionDecoderV2 (B=128, T=256, H=1024, V=32000).

Strategy (8 NeuronCores, one chip):
  - Attention: data-parallel over batch (16 rows/core). enc streamed once as
    [128=(t,b), 1024=H] bf16 tiles; scores via DVE tensor_tensor_reduce
    against a broadcast vat row; softmax fused into the single pass using a
    safe upper bound C = sum(|vat_bf16|) so exp(s-C)-weighted enc accumulates
    in PSUM through a constant 0/1 batch-selector matmul (contracts the
    (t,b)-partition axis down to the 16 local batch rows).
  - Combine + GRU: tensor-parallel, output-split over H (128 cols/core).
    Activations are kept transposed ([feature, batch]) so every matmul
    operand DMAs contiguously; small AllGathers (bf16) stitch the full
    transposed activations between stages.
  - Output projection + log_softmax: tensor-parallel over V (4000/core),
    host-pretransposed out_w; sharded logsumexp with one tiny AllGather of
    per-shard (max, sumexp).

kernel(**inputs) accepts the FULL unsharded inputs and returns the full
(logp, h_new, attn_w) tuple, sharding internally.
"""

import numpy as np
import ml_dtypes

import concourse.bacc as bacc
import concourse.mybir as mybir
import concourse.tile as tile
from concourse.bass_utils import run_bass_kernel_spmd

BF16 = ml_dtypes.bfloat16
F32 = mybir.dt.float32
BF = mybir.dt.bfloat16

B, T, H, V = 128, 256, 1024, 32000
NC = 8
BL = B // NC          # 16 batch rows per core (attention shard)
HL = H // NC          # 128 hidden cols per core (GRU/combine shard)
VL = V // NC          # 4000 vocab rows per core (out-proj shard)
NT = (T * BL) // 128  # 32 enc tiles of [128, H] per core
VCH = 8               # vocab chunks per core
VCW = VL // VCH       # 500 cols per vocab chunk
RG = [list(range(NC))]

_BUILD_CACHE = {}


def _build(repeat=1, phases=4):
    """Build + compile the SPMD Bass program (same program on all 8 cores)."""
    key = (repeat, phases)
    if key in _BUILD_CACHE:
        return _BUILD_CACHE[key]
    nc = bacc.Bacc("TRN2", target_bir_lowering=False, debug=False,
                   num_devices=NC)

    d = {}

    def din(name, shape, dt):
        d[name] = nc.dram_tensor(name, shape, dt, kind="ExternalInput").ap()

    def dout(name, shape, dt):
        d[name] = nc.dram_tensor(name, shape, dt, kind="ExternalOutput").ap()

    din("enc_sh", [T * BL, H], BF)        # per-core enc, (t,b)-major rows
    din("hidden_rep", [128, H], BF)       # local hidden rows tiled 8x
    din("vat_rep", [128, H], BF)          # vat_w broadcast to 128 partitions
    din("msk_bf", [128, BL], BF)          # msk[p,m] = (p%16==m)
    din("msk_f32", [128, BL], F32)
    din("negC", [128, 1], F32)            # -sum(|vat_bf16|)
    din("embT", [H, B], BF)               # gathered embedding, transposed
    din("hT_bf", [H, B], BF)              # hidden[0].T
    din("hT_f32_sl", [HL, B], F32)        # this core's rows of hidden.T (f32)
    din("comb_wT_sl", [2 * H, HL], BF)    # comb_w.T column slice
    din("comb_b_sl", [HL, 1], F32)
    din("wihT_sl", [H, 3 * HL], BF)       # gru_w_ih.T cols (r,z,n slices)
    din("whhT_sl", [H, 3 * HL], BF)
    din("gbias", [HL, 4], F32)            # r_comb, z_comb, b_ih_n, b_hh_n
    din("out_wT_sl", [H, VL], BF)         # out_w.T column slice
    din("out_b_sl", [1, VL], F32)
    din("ones1", [1, 128], F32)

    dout("logp_loc", [B, VL], F32)
    dout("hnewT_loc", [HL, B], F32)
    dout("attnw_loc", [BL, T], F32)

    AF = mybir.ActivationFunctionType
    OP = mybir.AluOpType
    AX = mybir.AxisListType

    with tile.TileContext(nc) as tc:
        for rep in range(repeat):
            _emit_once(nc, tc, d, AF, OP, AX, phases)
    nc.compile()
    _BUILD_CACHE[key] = (nc, d)
    return nc, d


def _emit_once(nc, tc, d, AF, OP, AX, phases=4):
    with (
        tc.tile_pool(name="const", bufs=1) as cp,
        tc.tile_pool(name="encp", bufs=4) as encp,
        tc.tile_pool(name="workp", bufs=3) as wkp,
        tc.tile_pool(name="smallp", bufs=2) as sp,
        tc.tile_pool(name="persist", bufs=1) as pp,
        tc.tile_pool(name="wpool", bufs=1) as wp,
        tc.tile_pool(name="owpool", bufs=24) as owp,
        tc.tile_pool(name="dram", bufs=1, space="DRAM") as dr,
    ):
        # ---- constants ----
        hid_t = cp.tile([128, H], BF, tag="hid")
        nc.sync.dma_start(hid_t[:], d["hidden_rep"])
        vat_t = cp.tile([128, H], BF, tag="vat")
        nc.sync.dma_start(vat_t[:], d["vat_rep"])
        mskb_t = cp.tile([128, BL], BF, tag="mskb")
        nc.sync.dma_start(mskb_t[:], d["msk_bf"])
        mskf_t = cp.tile([128, BL], F32, tag="mskf")
        nc.sync.dma_start(mskf_t[:], d["msk_f32"])
        negc_t = cp.tile([128, 1], F32, tag="negc")
        nc.sync.dma_start(negc_t[:], d["negC"])

        # =============== Phase A: attention (B-sharded) ===============
        E2 = pp.tile([128, NT], F32, tag="E2")  # exp(s - C), col per tile
        with tc.tile_pool(name="psA", bufs=1, space="PSUM") as psA:
            ps_a0 = psA.tile([BL, 512], F32, tag="a0")
            ps_a1 = psA.tile([BL, 512], F32, tag="a1")
            for j in range(NT):
                enc_t = encp.tile([128, H], BF, tag="enc")
                nc.sync.dma_start(enc_t[:], d["enc_sh"][j * 128:(j + 1) * 128, :])
                al = wkp.tile([128, H], BF, tag="al")
                nc.vector.tensor_add(al[:], enc_t[:], hid_t[:])
                al2 = wkp.tile([128, H], BF, tag="al2")
                nc.scalar.activation(al2[:], al[:], AF.Tanh)
                # NOTE: tensor_tensor_reduce faults the device on this
                # runtime (NRT_EXEC_UNIT_UNRECOVERABLE) — use mul + reduce.
                scr = wkp.tile([128, H], F32, tag="scr")
                nc.vector.tensor_mul(scr[:], al2[:], vat_t[:])
                nc.vector.tensor_reduce(E2[:, j:j + 1], scr[:],
                                        axis=AX.X, op=OP.add)
                nc.scalar.activation(E2[:, j:j + 1], E2[:, j:j + 1], AF.Exp,
                                     bias=negc_t[:, 0:1], scale=1.0)
                wj = sp.tile([128, BL], BF, tag="wj")
                nc.vector.tensor_scalar_mul(wj[:], mskb_t[:], E2[:, j:j + 1])
                nc.tensor.matmul(ps_a0[:], wj[:], enc_t[:, 0:512],
                                 start=(j == 0), stop=(j == NT - 1))
                nc.tensor.matmul(ps_a1[:], wj[:], enc_t[:, 512:1024],
                                 start=(j == 0), stop=(j == NT - 1))

            # normalization: Z[b] = sum_p msk[p,b] * E2[p, :] summed over tiles
            with tc.tile_pool(name="psZ", bufs=1, space="PSUM") as psZ:
                ps_z = psZ.tile([BL, NT], F32, tag="z")
                nc.tensor.matmul(ps_z[:], mskf_t[:], E2[:])
                z_s = sp.tile([BL, 1], F32, tag="zs")
                nc.vector.tensor_reduce(z_s[:], ps_z[:], axis=AX.X, op=OP.add)
            zinv = sp.tile([BL, 1], F32, tag="zinv")
            nc.vector.reciprocal(zinv[:], z_s[:])

            attn_bf = pp.tile([BL, H], BF, tag="attnbf")
            nc.vector.tensor_scalar_mul(attn_bf[:, 0:512], ps_a0[:], zinv[:, 0:1])
            nc.vector.tensor_scalar_mul(attn_bf[:, 512:1024], ps_a1[:], zinv[:, 0:1])

        # attn_w output: E2 [p=(t,b)%128, j] -> [b, t] via DRAM bounce
        e_dram = dr.tile([T * BL, 1], F32, tag="edram")
        nc.sync.dma_start(e_dram.rearrange("(j p) o -> p (j o)", p=128), E2[:])
        aw_raw = sp.tile([BL, T], F32, tag="awraw")
        nc.sync.dma_start(aw_raw[:], e_dram.rearrange("(t b) o -> b (t o)", b=BL))
        aw = sp.tile([BL, T], F32, tag="aw")
        nc.vector.tensor_scalar_mul(aw[:], aw_raw[:], zinv[:, 0:1])
        nc.sync.dma_start(d["attnw_loc"], aw[:])
        if phases < 2:
            return

        # =============== AG1: attn_applied [16,H] -> [128,H] ===============
        ag1_in = dr.tile([BL, H], BF, tag="ag1i")
        nc.gpsimd.dma_start(ag1_in[:], attn_bf[:])
        ag1_out = dr.tile([B, H], BF, tag="ag1o")
        nc.gpsimd.collective_compute(
            "AllGather", mybir.AluOpType.bypass, replica_groups=RG,
            ins=[ag1_in.opt()], outs=[ag1_out.opt()])

        # =============== Phase B: combine (TP over H cols) ===============
        with tc.tile_pool(name="psB", bufs=1, space="PSUM") as psB:
            ps_x = psB.tile([HL, B], F32, tag="x")
            for i in range(8):
                cw = wp.tile([128, HL], BF, tag=f"cw{i}")
                nc.sync.dma_start(cw[:], d["comb_wT_sl"][i * 128:(i + 1) * 128, :])
                et = wp.tile([128, B], BF, tag=f"et{i}")
                nc.sync.dma_start(et[:], d["embT"][i * 128:(i + 1) * 128, :])
                nc.tensor.matmul(ps_x[:], cw[:], et[:],
                                 start=(i == 0), stop=False)
            for i in range(8):
                cw2 = wp.tile([128, HL], BF, tag=f"cw2{i}")
                nc.sync.dma_start(cw2[:],
                                  d["comb_wT_sl"][H + i * 128:H + (i + 1) * 128, :])
                at = wp.tile([128, B], BF, tag=f"at{i}")
                nc.sync.dma_start_transpose(at[:], ag1_out[:, i * 128:(i + 1) * 128])
                nc.tensor.matmul(ps_x[:], cw2[:], at[:],
                                 start=False, stop=(i == 7))
            combb_t = cp.tile([HL, 1], F32, tag="combb")
            nc.sync.dma_start(combb_t[:], d["comb_b_sl"])
            xT_bf = pp.tile([HL, B], BF, tag="xT")
            nc.scalar.activation(xT_bf[:], ps_x[:], AF.Relu,
                                 bias=combb_t[:, 0:1], scale=1.0)

        # =============== AG2: xT [128,B] -> [H,B] ===============
        ag2_in = dr.tile([HL, B], BF, tag="ag2i")
        nc.gpsimd.dma_start(ag2_in[:], xT_bf[:])
        ag2_out = dr.tile([H, B], BF, tag="ag2o")
        nc.gpsimd.collective_compute(
            "AllGather", mybir.AluOpType.bypass, replica_groups=RG,
            ins=[ag2_in.opt()], outs=[ag2_out.opt()])
        if phases < 3:
            xdump = sp.tile([HL, B], F32, tag="xdump")
            nc.vector.tensor_copy(xdump[:], xT_bf[:])
            nc.sync.dma_start(d["hnewT_loc"], xdump[:])
            return

        # =============== Phase C: GRU (TP over H cols) ===============
        gb_t = cp.tile([HL, 4], F32, tag="gb")
        nc.sync.dma_start(gb_t[:], d["gbias"])
        with tc.tile_pool(name="psC", bufs=1, space="PSUM") as psC:
            ps_r = psC.tile([HL, B], F32, tag="r")
            ps_zg = psC.tile([HL, B], F32, tag="zg")
            ps_in = psC.tile([HL, B], F32, tag="in")
            ps_hn = psC.tile([HL, B], F32, tag="hn")
            for i in range(8):
                xt = wp.tile([128, B], BF, tag=f"xt{i}")
                nc.sync.dma_start(xt[:], ag2_out[i * 128:(i + 1) * 128, :])
                ht = wp.tile([128, B], BF, tag=f"ht{i}")
                nc.sync.dma_start(ht[:], d["hT_bf"][i * 128:(i + 1) * 128, :])
                ksl = slice(i * 128, (i + 1) * 128)
                wr = wp.tile([128, HL], BF, tag=f"wr{i}")
                nc.sync.dma_start(wr[:], d["wihT_sl"][ksl, 0:HL])
                nc.tensor.matmul(ps_r[:], wr[:], xt[:], start=(i == 0), stop=False)
                wr2 = wp.tile([128, HL], BF, tag=f"wr2{i}")
                nc.sync.dma_start(wr2[:], d["whhT_sl"][ksl, 0:HL])
                nc.tensor.matmul(ps_r[:], wr2[:], ht[:], start=False, stop=(i == 7))
                wz = wp.tile([128, HL], BF, tag=f"wz{i}")
                nc.sync.dma_start(wz[:], d["wihT_sl"][ksl, HL:2 * HL])
                nc.tensor.matmul(ps_zg[:], wz[:], xt[:], start=(i == 0), stop=False)
                wz2 = wp.tile([128, HL], BF, tag=f"wz2{i}")
                nc.sync.dma_start(wz2[:], d["whhT_sl"][ksl, HL:2 * HL])
                nc.tensor.matmul(ps_zg[:], wz2[:], ht[:], start=False, stop=(i == 7))
                wn = wp.tile([128, HL], BF, tag=f"wn{i}")
                nc.sync.dma_start(wn[:], d["wihT_sl"][ksl, 2 * HL:3 * HL])
                nc.tensor.matmul(ps_in[:], wn[:], xt[:],
                                 start=(i == 0), stop=(i == 7))
                wn2 = wp.tile([128, HL], BF, tag=f"wn2{i}")
                nc.sync.dma_start(wn2[:], d["whhT_sl"][ksl, 2 * HL:3 * HL])
                nc.tensor.matmul(ps_hn[:], wn2[:], ht[:],
                                 start=(i == 0), stop=(i == 7))

            rT = sp.tile([HL, B], F32, tag="rT")
            nc.scalar.activation(rT[:], ps_r[:], AF.Sigmoid, bias=gb_t[:, 0:1])
            zT = sp.tile([HL, B], F32, tag="zT")
            nc.scalar.activation(zT[:], ps_zg[:], AF.Sigmoid, bias=gb_t[:, 1:2])
            hnT = sp.tile([HL, B], F32, tag="hnT")
            nc.scalar.activation(hnT[:], ps_hn[:], AF.Identity, bias=gb_t[:, 3:4])
            rhn = sp.tile([HL, B], F32, tag="rhn")
            nc.vector.tensor_mul(rhn[:], rT[:], hnT[:])
            inb = sp.tile([HL, B], F32, tag="inb")
            nc.vector.tensor_scalar_add(inb[:], ps_in[:], gb_t[:, 2:3])
        nsum = sp.tile([HL, B], F32, tag="nsum")
        nc.vector.tensor_add(nsum[:], inb[:], rhn[:])
        nT = sp.tile([HL, B], F32, tag="nT")
        nc.scalar.activation(nT[:], nsum[:], AF.Tanh)
        hf_t = cp.tile([HL, B], F32, tag="hf")
        nc.sync.dma_start(hf_t[:], d["hT_f32_sl"])
        dhn = sp.tile([HL, B], F32, tag="dhn")
        nc.vector.tensor_sub(dhn[:], hf_t[:], nT[:])
        zd = sp.tile([HL, B], F32, tag="zd")
        nc.vector.tensor_mul(zd[:], zT[:], dhn[:])
        hnewT = sp.tile([HL, B], F32, tag="hnewT")
        nc.vector.tensor_add(hnewT[:], nT[:], zd[:])
        nc.sync.dma_start(d["hnewT_loc"], hnewT[:])
        hnew_bf = pp.tile([HL, B], BF, tag="hnewbf")
        nc.vector.tensor_copy(hnew_bf[:], hnewT[:])

        # =============== AG3: h_newT [128,B] -> [H,B] ===============
        ag3_in = dr.tile([HL, B], BF, tag="ag3i")
        nc.gpsimd.dma_start(ag3_in[:], hnew_bf[:])
        ag3_out = dr.tile([H, B], BF, tag="ag3o")
        nc.gpsimd.collective_compute(
            "AllGather", mybir.AluOpType.bypass, replica_groups=RG,
            ins=[ag3_in.opt()], outs=[ag3_out.opt()])
        if phases < 4:
            return

        # =============== Phase D: out proj + log_softmax (TP over V) =====
        ones_t = cp.tile([1, 128], F32, tag="ones")
        nc.sync.dma_start(ones_t[:], d["ones1"])
        outb_t = cp.tile([1, VL], F32, tag="outb")
        nc.sync.dma_start(outb_t[:], d["out_b_sl"])
        hn_ts = []
        for i in range(8):
            hnc = wp.tile([128, B], BF, tag=f"hnc{i}")
            nc.sync.dma_start(hnc[:], ag3_out[i * 128:(i + 1) * 128, :])
            hn_ts.append(hnc)
        logits = pp.tile([B, VL], F32, tag="logits")
        M8 = sp.tile([B, VCH], F32, tag="M8")
        S8 = sp.tile([B, VCH], F32, tag="S8")
        with tc.tile_pool(name="psD", bufs=2, space="PSUM") as psD:
            for c in range(VCH):
                csl = slice(c * VCW, (c + 1) * VCW)
                ps_l = psD.tile([B, VCW], F32, tag="l")
                for i in range(8):
                    wt = owp.tile([128, VCW], BF, tag="ow")
                    nc.sync.dma_start(
                        wt[:], d["out_wT_sl"][i * 128:(i + 1) * 128, csl])
                    nc.tensor.matmul(ps_l[:], hn_ts[i][:], wt[:],
                                     start=(i == 0), stop=False)
                nc.tensor.matmul(ps_l[:], ones_t[:], outb_t[:, csl],
                                 start=False, stop=True)
                nc.vector.tensor_copy(logits[:, csl], ps_l[:])
                nc.vector.tensor_reduce(M8[:, c:c + 1], ps_l[:],
                                        axis=AX.X, op=OP.max)
        m_loc = sp.tile([B, 1], F32, tag="mloc")
        nc.vector.tensor_reduce(m_loc[:], M8[:], axis=AX.X, op=OP.max)
        negm = sp.tile([B, 1], F32, tag="negm")
        nc.vector.tensor_scalar_mul(negm[:], m_loc[:], -1.0)
        for c in range(VCH):
            csl = slice(c * VCW, (c + 1) * VCW)
            esc = wkp.tile([B, VCW], F32, tag="esc")
            nc.scalar.activation(esc[:], logits[:, csl], AF.Exp,
                                 bias=negm[:, 0:1], accum_out=S8[:, c:c + 1])
        s_loc = sp.tile([B, 1], F32, tag="sloc")
        nc.vector.tensor_reduce(s_loc[:], S8[:], axis=AX.X, op=OP.add)

        # tiny AG of (m, S) pairs
        p2 = sp.tile([B, 2], F32, tag="p2")
        nc.vector.tensor_copy(p2[:, 0:1], m_loc[:])
        nc.vector.tensor_copy(p2[:, 1:2], s_loc[:])
        ag4_in = dr.tile([B, 2], F32, tag="ag4i")
        nc.gpsimd.dma_start(ag4_in[:], p2[:])
        ag4_out = dr.tile([NC * B, 2], F32, tag="ag4o")
        nc.gpsimd.collective_compute(
            "AllGather", mybir.AluOpType.bypass, replica_groups=RG,
            ins=[ag4_in.opt()], outs=[ag4_out.opt()])
        m_all = sp.tile([B, NC], F32, tag="mall")
        nc.sync.dma_start(m_all[:],
                          ag4_out[:, 0:1].rearrange("(i b) o -> b (i o)", b=B))
        s_all = sp.tile([B, NC], F32, tag="sall")
        nc.sync.dma_start(s_all[:],
                          ag4_out[:, 1:2].rearrange("(i b) o -> b (i o)", b=B))
        mg = sp.tile([B, 1], F32, tag="mg")
        nc.vector.tensor_reduce(mg[:], m_all[:], axis=AX.X, op=OP.max)
        negmg = sp.tile([B, 1], F32, tag="negmg")
        nc.vector.tensor_scalar_mul(negmg[:], mg[:], -1.0)
        em = sp.tile([B, NC], F32, tag="em")
        nc.scalar.activation(em[:], m_all[:], AF.Exp, bias=negmg[:, 0:1])
        t8 = sp.tile([B, NC], F32, tag="t8")
        nc.vector.tensor_mul(t8[:], em[:], s_all[:])
        sg = sp.tile([B, 1], F32, tag="sg")
        nc.vector.tensor_reduce(sg[:], t8[:], axis=AX.X, op=OP.add)
        lns = sp.tile([B, 1], F32, tag="lns")
        nc.scalar.activation(lns[:], sg[:], AF.Ln)
        lse = sp.tile([B, 1], F32, tag="lse")
        nc.vector.tensor_add(lse[:], mg[:], lns[:])
        neglse = sp.tile([B, 1], F32, tag="neglse")
        nc.vector.tensor_scalar_mul(neglse[:], lse[:], -1.0)
        for c in range(VCH):
            csl = slice(c * VCW, (c + 1) * VCW)
            lp = wkp.tile([B, VCW], F32, tag="lp")
            nc.scalar.activation(lp[:], logits[:, csl], AF.Identity,
                                 bias=neglse[:, 0:1])
            nc.sync.dma_start(d["logp_loc"][:, csl], lp[:])


def _prepare_in_maps(input_ids, hidden, enc, emb, vat_w, vat_b, comb_w,
                     comb_b, gru_w_ih, gru_w_hh, gru_b_ih, gru_b_hh,
                     out_w, out_b):
    f32 = np.float32
    hidden = np.asarray(hidden, f32)
    enc = np.asarray(enc, f32)
    ids = np.asarray(input_ids).astype(np.int64)

    embedded = np.asarray(emb, f32)[ids]                     # [B, H]
    embT = np.ascontiguousarray(embedded.T).astype(BF16)     # [H, B]
    hiddenT = np.ascontiguousarray(hidden[0].T.astype(f32))  # [H, B]
    hT_bf = hiddenT.astype(BF16)

    vat = np.asarray(vat_w, f32).reshape(H)
    vat_bf = vat.astype(BF16)
    vat_rep = np.tile(vat_bf.reshape(1, H), (128, 1))
    # |score| <= sum|vat|; exp only needs a shift if that bound could
    # overflow f32 (e^60). Keeping the shift minimal keeps exp() arguments
    # near 0 where the ACT LUT is most accurate.
    C = max(0.0, float(np.abs(vat_bf.astype(np.float64)).sum()) - 60.0)
    negC = np.full((128, 1), -C, f32)
    msk = (np.arange(128)[:, None] % BL == np.arange(BL)[None, :])
    msk_bf = msk.astype(BF16)
    msk_f32 = msk.astype(f32)

    comb_wT = np.asarray(comb_w, f32).T                      # [2H, H]
    gwihT = np.asarray(gru_w_ih, f32).T                      # [H, 3H]
    gwhhT = np.asarray(gru_w_hh, f32).T
    b_ih = np.asarray(gru_b_ih, f32)
    b_hh = np.asarray(gru_b_hh, f32)
    out_w = np.asarray(out_w, f32)
    out_b = np.asarray(out_b, f32)
    comb_b = np.asarray(comb_b, f32)
    ones1 = np.ones((1, 128), f32)

    in_maps = []
    for i in range(NC):
        b0 = i * BL
        ssl = slice(i * HL, (i + 1) * HL)
        vsl = slice(i * VL, (i + 1) * VL)
        enc_sh = np.ascontiguousarray(
            enc[:, b0:b0 + BL, :].reshape(T * BL, H)).astype(BF16)
        hidden_rep = np.tile(hidden[0, b0:b0 + BL, :], (128 // BL, 1)).astype(BF16)
        gcols = np.concatenate([np.arange(ssl.start, ssl.stop),
                                H + np.arange(ssl.start, ssl.stop),
                                2 * H + np.arange(ssl.start, ssl.stop)])
        gbias = np.stack([
            b_ih[ssl] + b_hh[ssl],
            b_ih[H + i * HL:H + (i + 1) * HL] + b_hh[H + i * HL:H + (i + 1) * HL],
            b_ih[2 * H + i * HL:2 * H + (i + 1) * HL],
            b_hh[2 * H + i * HL:2 * H + (i + 1) * HL],
        ], axis=1).astype(f32)                               # [HL, 4]
        in_maps.append({
            "enc_sh": enc_sh,
            "hidden_rep": hidden_rep,
            "vat_rep": vat_rep,
            "msk_bf": msk_bf,
            "msk_f32": msk_f32,
            "negC": negC,
            "embT": embT,
            "hT_bf": hT_bf,
            "hT_f32_sl": np.ascontiguousarray(hiddenT[ssl, :]),
            "comb_wT_sl": np.ascontiguousarray(comb_wT[:, ssl]).astype(BF16),
            "comb_b_sl": comb_b[ssl].reshape(HL, 1),
            "wihT_sl": np.ascontiguousarray(gwihT[:, gcols]).astype(BF16),
            "whhT_sl": np.ascontiguousarray(gwhhT[:, gcols]).astype(BF16),
            "gbias": gbias,
            "out_wT_sl": np.ascontiguousarray(out_w[vsl, :].T).astype(BF16),
            "out_b_sl": out_b[vsl].reshape(1, VL),
            "ones1": ones1,
        })
    return in_maps


def _assemble(results):
    logp = np.concatenate([results[i]["logp_loc"] for i in range(NC)], axis=1)
    h_new = np.concatenate([results[i]["hnewT_loc"].T for i in range(NC)],
                           axis=1)[None]
    attn_w = np.concatenate([results[i]["attnw_loc"] for i in range(NC)],
                            axis=0)[:, None, :]
    return logp.astype(np.float32), h_new.astype(np.float32), \
        attn_w.astype(np.float32)


def kernel(**inputs):
    nc, _ = _build(repeat=1)
    in_maps = _prepare_in_maps(**inputs)
    res = run_bass_kernel_spmd(nc, in_maps, core_ids=list(range(NC)))
    return _assemble(res.results)


# revision 11
# speedup vs baseline: 1.1801x; 1.1801x over previous
"""Trainium2 Bass kernel for AttentionDecoderV2 (B=128, T=256, H=1024, V=32000).

Strategy (8 NeuronCores, one chip):
  - Attention: data-parallel over batch (16 rows/core). enc streamed once as
    [128=(t,b), 1024=H] bf16 tiles; scores via DVE tensor_tensor_reduce
    against a broadcast vat row; softmax fused into the single pass using a
    safe upper bound C = sum(|vat_bf16|) so exp(s-C)-weighted enc accumulates
    in PSUM through a constant 0/1 batch-selector matmul (contracts the
    (t,b)-partition axis down to the 16 local batch rows).
  - Combine + GRU: tensor-parallel, output-split over H (128 cols/core).
    Activations are kept transposed ([feature, batch]) so every matmul
    operand DMAs contiguously; small AllGathers (bf16) stitch the full
    transposed activations between stages.
  - Output projection + log_softmax: tensor-parallel over V (4000/core),
    host-pretransposed out_w; sharded logsumexp with one tiny AllGather of
    per-shard (max, sumexp).

kernel(**inputs) accepts the FULL unsharded inputs and returns the full
(logp, h_new, attn_w) tuple, sharding internally.
"""

import numpy as np
import ml_dtypes

import concourse.bacc as bacc
import concourse.mybir as mybir
import concourse.tile as tile
from concourse.bass_utils import run_bass_kernel_spmd

BF16 = ml_dtypes.bfloat16
F32 = mybir.dt.float32
BF = mybir.dt.bfloat16

B, T, H, V = 128, 256, 1024, 32000
NC = 8
BL = B // NC          # 16 batch rows per core (attention shard)
HL = H // NC          # 128 hidden cols per core (GRU/combine shard)
VL = V // NC          # 4000 vocab rows per core (out-proj shard)
NT = (T * BL) // 128  # 32 enc tiles of [128, H] per core
VCH = 8               # vocab chunks per core
VCW = VL // VCH       # 500 cols per vocab chunk
RG = [list(range(NC))]

_BUILD_CACHE = {}


def _build(repeat=1, phases=4):
    """Build + compile the SPMD Bass program (same program on all 8 cores)."""
    key = (repeat, phases)
    if key in _BUILD_CACHE:
        return _BUILD_CACHE[key]
    nc = bacc.Bacc("TRN2", target_bir_lowering=False, debug=False,
                   num_devices=NC)

    d = {}

    def din(name, shape, dt):
        d[name] = nc.dram_tensor(name, shape, dt, kind="ExternalInput").ap()

    def dout(name, shape, dt):
        d[name] = nc.dram_tensor(name, shape, dt, kind="ExternalOutput").ap()

    din("enc_sh", [T * BL, H], BF)        # per-core enc, (t,b)-major rows
    din("hidden_rep", [128, H], BF)       # local hidden rows tiled 8x
    din("vat_rep", [128, H], BF)          # vat_w broadcast to 128 partitions
    din("msk_bf", [128, BL], BF)          # msk[p,m] = (p%16==m)
    din("msk_f32", [128, BL], F32)
    din("negC", [128, 1], F32)            # -sum(|vat_bf16|)
    din("embT", [H, B], BF)               # gathered embedding, transposed
    din("hT_bf", [H, B], BF)              # hidden[0].T
    din("hT_f32_sl", [HL, B], F32)        # this core's rows of hidden.T (f32)
    din("comb_wT_sl", [2 * H, HL], BF)    # comb_w.T column slice
    din("comb_b_sl", [HL, 1], F32)
    din("wihT_sl", [H, 3 * HL], BF)       # gru_w_ih.T cols (r,z,n slices)
    din("whhT_sl", [H, 3 * HL], BF)
    din("gbias", [HL, 4], F32)            # r_comb, z_comb, b_ih_n, b_hh_n
    din("out_wT_sl", [H, VL], BF)         # out_w.T column slice
    din("out_b_sl", [1, VL], F32)
    din("ones1", [1, 128], F32)

    dout("logp_loc", [B, VL], F32)
    dout("hnewT_loc", [HL, B], F32)
    dout("attnw_loc", [BL, T], F32)

    AF = mybir.ActivationFunctionType
    OP = mybir.AluOpType
    AX = mybir.AxisListType

    with tile.TileContext(nc) as tc:
        for rep in range(repeat):
            _emit_once(nc, tc, d, AF, OP, AX, phases)
    nc.compile()
    _BUILD_CACHE[key] = (nc, d)
    return nc, d


def _emit_once(nc, tc, d, AF, OP, AX, phases=4):
    with (
        tc.tile_pool(name="const", bufs=1) as cp,
        tc.tile_pool(name="encp", bufs=6) as encp,
        tc.tile_pool(name="workp", bufs=3) as wkp,
        tc.tile_pool(name="smallp", bufs=2) as sp,
        tc.tile_pool(name="persist", bufs=1) as pp,
        tc.tile_pool(name="wpool", bufs=1) as wp,
        tc.tile_pool(name="owpool", bufs=56) as owp,
        tc.tile_pool(name="dram", bufs=1, space="DRAM") as dr,
    ):
        # ---- constants ----
        hid_t = cp.tile([128, H], BF, tag="hid")
        nc.sync.dma_start(hid_t[:], d["hidden_rep"])
        vat_t = cp.tile([128, H], BF, tag="vat")
        nc.sync.dma_start(vat_t[:], d["vat_rep"])
        mskb_t = cp.tile([128, BL], BF, tag="mskb")
        nc.sync.dma_start(mskb_t[:], d["msk_bf"])
        mskf_t = cp.tile([128, BL], F32, tag="mskf")
        nc.sync.dma_start(mskf_t[:], d["msk_f32"])
        negc_t = cp.tile([128, 1], F32, tag="negc")
        nc.sync.dma_start(negc_t[:], d["negC"])

        # =============== Phase A: attention (B-sharded) ===============
        E2 = pp.tile([128, NT], F32, tag="E2")  # exp(s - C), col per tile
        with tc.tile_pool(name="psA", bufs=1, space="PSUM") as psA:
            ps_a0 = psA.tile([BL, 512], F32, tag="a0")
            ps_a1 = psA.tile([BL, 512], F32, tag="a1")
            for j in range(NT):
                enc_t = encp.tile([128, H], BF, tag="enc")
                nc.sync.dma_start(enc_t[:], d["enc_sh"][j * 128:(j + 1) * 128, :])
                al = wkp.tile([128, H], BF, tag="al")
                nc.vector.tensor_add(al[:], enc_t[:], hid_t[:])
                al2 = wkp.tile([128, H], BF, tag="al2")
                nc.scalar.activation(al2[:], al[:], AF.Tanh)
                # NOTE: tensor_tensor_reduce faults the device on this
                # runtime (NRT_EXEC_UNIT_UNRECOVERABLE) — split it. All-bf16
                # keeps DVE in 2x mode; one fold halves the sum length, and
                # the final sum rides ScalarE's accum_out so DVE and ACT
                # stay balanced.
                scr = wkp.tile([128, H], BF, tag="scr")
                nc.vector.tensor_mul(scr[:], al2[:], vat_t[:])
                scrf = wkp.tile([128, 512], BF, tag="scrf")
                nc.vector.tensor_add(scrf[:], scr[:, 0:512], scr[:, 512:1024])
                scrd = wkp.tile([128, 512], BF, tag="scrd")
                nc.scalar.activation(scrd[:], scrf[:], AF.Identity,
                                     accum_out=E2[:, j:j + 1])
                nc.scalar.activation(E2[:, j:j + 1], E2[:, j:j + 1], AF.Exp,
                                     bias=negc_t[:, 0:1], scale=1.0)
                wj = sp.tile([128, BL], BF, tag="wj")
                nc.vector.tensor_scalar_mul(wj[:], mskb_t[:], E2[:, j:j + 1])
                nc.tensor.matmul(ps_a0[:], wj[:], enc_t[:, 0:512],
                                 start=(j == 0), stop=(j == NT - 1))
                nc.tensor.matmul(ps_a1[:], wj[:], enc_t[:, 512:1024],
                                 start=(j == 0), stop=(j == NT - 1))

            # normalization: Z[b] = sum_p msk[p,b] * E2[p, :] summed over tiles
            with tc.tile_pool(name="psZ", bufs=1, space="PSUM") as psZ:
                ps_z = psZ.tile([BL, NT], F32, tag="z")
                nc.tensor.matmul(ps_z[:], mskf_t[:], E2[:])
                z_s = sp.tile([BL, 1], F32, tag="zs")
                nc.vector.tensor_reduce(z_s[:], ps_z[:], axis=AX.X, op=OP.add)
            zinv = sp.tile([BL, 1], F32, tag="zinv")
            nc.vector.reciprocal(zinv[:], z_s[:])

            attn_bf = pp.tile([BL, H], BF, tag="attnbf")
            nc.vector.tensor_scalar_mul(attn_bf[:, 0:512], ps_a0[:], zinv[:, 0:1])
            nc.vector.tensor_scalar_mul(attn_bf[:, 512:1024], ps_a1[:], zinv[:, 0:1])

        # attn_w output: E2 [p=(t,b)%128, j] -> [b, t] via DRAM bounce
        e_dram = dr.tile([T * BL, 1], F32, tag="edram")
        nc.sync.dma_start(e_dram.rearrange("(j p) o -> p (j o)", p=128), E2[:])
        aw_raw = sp.tile([BL, T], F32, tag="awraw")
        nc.sync.dma_start(aw_raw[:], e_dram.rearrange("(t b) o -> b (t o)", b=BL))
        aw = sp.tile([BL, T], F32, tag="aw")
        nc.vector.tensor_scalar_mul(aw[:], aw_raw[:], zinv[:, 0:1])
        nc.sync.dma_start(d["attnw_loc"], aw[:])
        if phases < 2:
            return

        # =============== AG1: attn_applied [16,H] -> [128,H] ===============
        ag1_in = dr.tile([BL, H], BF, tag="ag1i")
        nc.gpsimd.dma_start(ag1_in[:], attn_bf[:])
        ag1_out = dr.tile([B, H], BF, tag="ag1o")
        nc.gpsimd.collective_compute(
            "AllGather", mybir.AluOpType.bypass, replica_groups=RG,
            ins=[ag1_in.opt()], outs=[ag1_out.opt()])

        # =============== Phase B: combine (TP over H cols) ===============
        with tc.tile_pool(name="psB", bufs=1, space="PSUM") as psB:
            ps_x = psB.tile([HL, B], F32, tag="x")
            for i in range(8):
                cw = wp.tile([128, HL], BF, tag=f"cw{i}")
                nc.sync.dma_start(cw[:], d["comb_wT_sl"][i * 128:(i + 1) * 128, :])
                et = wp.tile([128, B], BF, tag=f"et{i}")
                nc.sync.dma_start(et[:], d["embT"][i * 128:(i + 1) * 128, :])
                nc.tensor.matmul(ps_x[:], cw[:], et[:],
                                 start=(i == 0), stop=False)
            for i in range(8):
                cw2 = wp.tile([128, HL], BF, tag=f"cw2{i}")
                nc.sync.dma_start(cw2[:],
                                  d["comb_wT_sl"][H + i * 128:H + (i + 1) * 128, :])
                at = wp.tile([128, B], BF, tag=f"at{i}")
                nc.sync.dma_start_transpose(at[:], ag1_out[:, i * 128:(i + 1) * 128])
                nc.tensor.matmul(ps_x[:], cw2[:], at[:],
                                 start=False, stop=(i == 7))
            combb_t = cp.tile([HL, 1], F32, tag="combb")
            nc.sync.dma_start(combb_t[:], d["comb_b_sl"])
            xT_bf = pp.tile([HL, B], BF, tag="xT")
            nc.scalar.activation(xT_bf[:], ps_x[:], AF.Relu,
                                 bias=combb_t[:, 0:1], scale=1.0)

        # =============== AG2: xT [128,B] -> [H,B] ===============
        ag2_in = dr.tile([HL, B], BF, tag="ag2i")
        nc.gpsimd.dma_start(ag2_in[:], xT_bf[:])
        ag2_out = dr.tile([H, B], BF, tag="ag2o")
        nc.gpsimd.collective_compute(
            "AllGather", mybir.AluOpType.bypass, replica_groups=RG,
            ins=[ag2_in.opt()], outs=[ag2_out.opt()])
        if phases < 3:
            xdump = sp.tile([HL, B], F32, tag="xdump")
            nc.vector.tensor_copy(xdump[:], xT_bf[:])
            nc.sync.dma_start(d["hnewT_loc"], xdump[:])
            return

        # =============== Phase C: GRU (TP over H cols) ===============
        gb_t = cp.tile([HL, 4], F32, tag="gb")
        nc.sync.dma_start(gb_t[:], d["gbias"])
        with tc.tile_pool(name="psC", bufs=1, space="PSUM") as psC:
            ps_r = psC.tile([HL, B], F32, tag="r")
            ps_zg = psC.tile([HL, B], F32, tag="zg")
            ps_in = psC.tile([HL, B], F32, tag="in")
            ps_hn = psC.tile([HL, B], F32, tag="hn")
            for i in range(8):
                xt = wp.tile([128, B], BF, tag=f"xt{i}")
                nc.sync.dma_start(xt[:], ag2_out[i * 128:(i + 1) * 128, :])
                ht = wp.tile([128, B], BF, tag=f"ht{i}")
                nc.sync.dma_start(ht[:], d["hT_bf"][i * 128:(i + 1) * 128, :])
                ksl = slice(i * 128, (i + 1) * 128)
                wr = wp.tile([128, HL], BF, tag=f"wr{i}")
                nc.sync.dma_start(wr[:], d["wihT_sl"][ksl, 0:HL])
                nc.tensor.matmul(ps_r[:], wr[:], xt[:], start=(i == 0), stop=False)
                wr2 = wp.tile([128, HL], BF, tag=f"wr2{i}")
                nc.sync.dma_start(wr2[:], d["whhT_sl"][ksl, 0:HL])
                nc.tensor.matmul(ps_r[:], wr2[:], ht[:], start=False, stop=(i == 7))
                wz = wp.tile([128, HL], BF, tag=f"wz{i}")
                nc.sync.dma_start(wz[:], d["wihT_sl"][ksl, HL:2 * HL])
                nc.tensor.matmul(ps_zg[:], wz[:], xt[:], start=(i == 0), stop=False)
                wz2 = wp.tile([128, HL], BF, tag=f"wz2{i}")
                nc.sync.dma_start(wz2[:], d["whhT_sl"][ksl, HL:2 * HL])
                nc.tensor.matmul(ps_zg[:], wz2[:], ht[:], start=False, stop=(i == 7))
                wn = wp.tile([128, HL], BF, tag=f"wn{i}")
                nc.sync.dma_start(wn[:], d["wihT_sl"][ksl, 2 * HL:3 * HL])
                nc.tensor.matmul(ps_in[:], wn[:], xt[:],
                                 start=(i == 0), stop=(i == 7))
                wn2 = wp.tile([128, HL], BF, tag=f"wn2{i}")
                nc.sync.dma_start(wn2[:], d["whhT_sl"][ksl, 2 * HL:3 * HL])
                nc.tensor.matmul(ps_hn[:], wn2[:], ht[:],
                                 start=(i == 0), stop=(i == 7))

            rT = sp.tile([HL, B], F32, tag="rT")
            nc.scalar.activation(rT[:], ps_r[:], AF.Sigmoid, bias=gb_t[:, 0:1])
            zT = sp.tile([HL, B], F32, tag="zT")
            nc.scalar.activation(zT[:], ps_zg[:], AF.Sigmoid, bias=gb_t[:, 1:2])
            hnT = sp.tile([HL, B], F32, tag="hnT")
            nc.scalar.activation(hnT[:], ps_hn[:], AF.Identity, bias=gb_t[:, 3:4])
            rhn = sp.tile([HL, B], F32, tag="rhn")
            nc.vector.tensor_mul(rhn[:], rT[:], hnT[:])
            inb = sp.tile([HL, B], F32, tag="inb")
            nc.vector.tensor_scalar_add(inb[:], ps_in[:], gb_t[:, 2:3])
        nsum = sp.tile([HL, B], F32, tag="nsum")
        nc.vector.tensor_add(nsum[:], inb[:], rhn[:])
        nT = sp.tile([HL, B], F32, tag="nT")
        nc.scalar.activation(nT[:], nsum[:], AF.Tanh)
        hf_t = cp.tile([HL, B], F32, tag="hf")
        nc.sync.dma_start(hf_t[:], d["hT_f32_sl"])
        dhn = sp.tile([HL, B], F32, tag="dhn")
        nc.vector.tensor_sub(dhn[:], hf_t[:], nT[:])
        zd = sp.tile([HL, B], F32, tag="zd")
        nc.vector.tensor_mul(zd[:], zT[:], dhn[:])
        hnewT = sp.tile([HL, B], F32, tag="hnewT")
        nc.vector.tensor_add(hnewT[:], nT[:], zd[:])
        nc.sync.dma_start(d["hnewT_loc"], hnewT[:])
        hnew_bf = pp.tile([HL, B], BF, tag="hnewbf")
        nc.vector.tensor_copy(hnew_bf[:], hnewT[:])

        # =============== AG3: h_newT [128,B] -> [H,B] ===============
        ag3_in = dr.tile([HL, B], BF, tag="ag3i")
        nc.gpsimd.dma_start(ag3_in[:], hnew_bf[:])
        ag3_out = dr.tile([H, B], BF, tag="ag3o")
        nc.gpsimd.collective_compute(
            "AllGather", mybir.AluOpType.bypass, replica_groups=RG,
            ins=[ag3_in.opt()], outs=[ag3_out.opt()])
        if phases < 4:
            return

        # =============== Phase D: out proj + log_softmax (TP over V) =====
        ones_t = cp.tile([1, 128], F32, tag="ones")
        nc.sync.dma_start(ones_t[:], d["ones1"])
        outb_t = cp.tile([1, VL], F32, tag="outb")
        nc.sync.dma_start(outb_t[:], d["out_b_sl"])
        hn_ts = []
        for i in range(8):
            hnc = wp.tile([128, B], BF, tag=f"hnc{i}")
            nc.sync.dma_start(hnc[:], ag3_out[i * 128:(i + 1) * 128, :])
            hn_ts.append(hnc)
        logits = pp.tile([B, VL], F32, tag="logits")
        M8 = sp.tile([B, VCH], F32, tag="M8")
        S8 = sp.tile([B, VCH], F32, tag="S8")
        with tc.tile_pool(name="psD", bufs=2, space="PSUM") as psD:
            for c in range(VCH):
                csl = slice(c * VCW, (c + 1) * VCW)
                ps_l = psD.tile([B, VCW], F32, tag="l")
                for i in range(8):
                    wt = owp.tile([128, VCW], BF, tag="ow")
                    nc.sync.dma_start(
                        wt[:], d["out_wT_sl"][i * 128:(i + 1) * 128, csl])
                    nc.tensor.matmul(ps_l[:], hn_ts[i][:], wt[:],
                                     start=(i == 0), stop=False)
                nc.tensor.matmul(ps_l[:], ones_t[:], outb_t[:, csl],
                                 start=False, stop=True)
                nc.vector.tensor_copy(logits[:, csl], ps_l[:])
                nc.vector.tensor_reduce(M8[:, c:c + 1], ps_l[:],
                                        axis=AX.X, op=OP.max)
        m_loc = sp.tile([B, 1], F32, tag="mloc")
        nc.vector.tensor_reduce(m_loc[:], M8[:], axis=AX.X, op=OP.max)
        negm = sp.tile([B, 1], F32, tag="negm")
        nc.vector.tensor_scalar_mul(negm[:], m_loc[:], -1.0)
        for c in range(VCH):
            csl = slice(c * VCW, (c + 1) * VCW)
            esc = wkp.tile([B, VCW], F32, tag="esc")
            nc.scalar.activation(esc[:], logits[:, csl], AF.Exp,
                                 bias=negm[:, 0:1], accum_out=S8[:, c:c + 1])
        s_loc = sp.tile([B, 1], F32, tag="sloc")
        nc.vector.tensor_reduce(s_loc[:], S8[:], axis=AX.X, op=OP.add)

        # tiny AG of (m, S) pairs
        p2 = sp.tile([B, 2], F32, tag="p2")
        nc.vector.tensor_copy(p2[:, 0:1], m_loc[:])
        nc.vector.tensor_copy(p2[:, 1:2], s_loc[:])
        ag4_in = dr.tile([B, 2], F32, tag="ag4i")
        nc.gpsimd.dma_start(ag4_in[:], p2[:])
        ag4_out = dr.tile([NC * B, 2], F32, tag="ag4o")
        nc.gpsimd.collective_compute(
            "AllGather", mybir.AluOpType.bypass, replica_groups=RG,
            ins=[ag4_in.opt()], outs=[ag4_out.opt()])
        m_all = sp.tile([B, NC], F32, tag="mall")
        nc.sync.dma_start(m_all[:],
                          ag4_out[:, 0:1].rearrange("(i b) o -> b (i o)", b=B))
        s_all = sp.tile([B, NC], F32, tag="sall")
        nc.sync.dma_start(s_all[:],
                          ag4_out[:, 1:2].rearrange("(i b) o -> b (i o)", b=B))
        mg = sp.tile([B, 1], F32, tag="mg")
        nc.vector.tensor_reduce(mg[:], m_all[:], axis=AX.X, op=OP.max)
        negmg = sp.tile([B, 1], F32, tag="negmg")
        nc.vector.tensor_scalar_mul(negmg[:], mg[:], -1.0)
        em = sp.tile([B, NC], F32, tag="em")
        nc.scalar.activation(em[:], m_all[:], AF.Exp, bias=negmg[:, 0:1])
        t8 = sp.tile([B, NC], F32, tag="t8")
        nc.vector.tensor_mul(t8[:], em[:], s_all[:])
        sg = sp.tile([B, 1], F32, tag="sg")
        nc.vector.tensor_reduce(sg[:], t8[:], axis=AX.X, op=OP.add)
        lns = sp.tile([B, 1], F32, tag="lns")
        nc.scalar.activation(lns[:], sg[:], AF.Ln)
        lse = sp.tile([B, 1], F32, tag="lse")
        nc.vector.tensor_add(lse[:], mg[:], lns[:])
        neglse = sp.tile([B, 1], F32, tag="neglse")
        nc.vector.tensor_scalar_mul(neglse[:], lse[:], -1.0)
        for c in range(VCH):
            csl = slice(c * VCW, (c + 1) * VCW)
            lp = wkp.tile([B, VCW], F32, tag="lp")
            nc.scalar.activation(lp[:], logits[:, csl], AF.Identity,
                                 bias=neglse[:, 0:1])
            nc.sync.dma_start(d["logp_loc"][:, csl], lp[:])


def _prepare_in_maps(input_ids, hidden, enc, emb, vat_w, vat_b, comb_w,
                     comb_b, gru_w_ih, gru_w_hh, gru_b_ih, gru_b_hh,
                     out_w, out_b):
    f32 = np.float32
    hidden = np.asarray(hidden, f32)
    enc = np.asarray(enc, f32)
    ids = np.asarray(input_ids).astype(np.int64)

    embedded = np.asarray(emb, f32)[ids]                     # [B, H]
    embT = np.ascontiguousarray(embedded.T).astype(BF16)     # [H, B]
    hiddenT = np.ascontiguousarray(hidden[0].T.astype(f32))  # [H, B]
    hT_bf = hiddenT.astype(BF16)

    vat = np.asarray(vat_w, f32).reshape(H)
    vat_bf = vat.astype(BF16)
    vat_rep = np.tile(vat_bf.reshape(1, H), (128, 1))
    # |score| <= sum|vat|; exp only needs a shift if that bound could
    # overflow f32 (e^60). Keeping the shift minimal keeps exp() arguments
    # near 0 where the ACT LUT is most accurate.
    C = max(0.0, float(np.abs(vat_bf.astype(np.float64)).sum()) - 60.0)
    negC = np.full((128, 1), -C, f32)
    msk = (np.arange(128)[:, None] % BL == np.arange(BL)[None, :])
    msk_bf = msk.astype(BF16)
    msk_f32 = msk.astype(f32)

    comb_wT = np.asarray(comb_w, f32).T                      # [2H, H]
    gwihT = np.asarray(gru_w_ih, f32).T                      # [H, 3H]
    gwhhT = np.asarray(gru_w_hh, f32).T
    b_ih = np.asarray(gru_b_ih, f32)
    b_hh = np.asarray(gru_b_hh, f32)
    out_w = np.asarray(out_w, f32)
    out_b = np.asarray(out_b, f32)
    comb_b = np.asarray(comb_b, f32)
    ones1 = np.ones((1, 128), f32)

    in_maps = []
    for i in range(NC):
        b0 = i * BL
        ssl = slice(i * HL, (i + 1) * HL)
        vsl = slice(i * VL, (i + 1) * VL)
        enc_sh = np.ascontiguousarray(
            enc[:, b0:b0 + BL, :].reshape(T * BL, H)).astype(BF16)
        hidden_rep = np.tile(hidden[0, b0:b0 + BL, :], (128 // BL, 1)).astype(BF16)
        gcols = np.concatenate([np.arange(ssl.start, ssl.stop),
                                H + np.arange(ssl.start, ssl.stop),
                                2 * H + np.arange(ssl.start, ssl.stop)])
        gbias = np.stack([
            b_ih[ssl] + b_hh[ssl],
            b_ih[H + i * HL:H + (i + 1) * HL] + b_hh[H + i * HL:H + (i + 1) * HL],
            b_ih[2 * H + i * HL:2 * H + (i + 1) * HL],
            b_hh[2 * H + i * HL:2 * H + (i + 1) * HL],
        ], axis=1).astype(f32)                               # [HL, 4]
        in_maps.append({
            "enc_sh": enc_sh,
            "hidden_rep": hidden_rep,
            "vat_rep": vat_rep,
            "msk_bf": msk_bf,
            "msk_f32": msk_f32,
            "negC": negC,
            "embT": embT,
            "hT_bf": hT_bf,
            "hT_f32_sl": np.ascontiguousarray(hiddenT[ssl, :]),
            "comb_wT_sl": np.ascontiguousarray(comb_wT[:, ssl]).astype(BF16),
            "comb_b_sl": comb_b[ssl].reshape(HL, 1),
            "wihT_sl": np.ascontiguousarray(gwihT[:, gcols]).astype(BF16),
            "whhT_sl": np.ascontiguousarray(gwhhT[:, gcols]).astype(BF16),
            "gbias": gbias,
            "out_wT_sl": np.ascontiguousarray(out_w[vsl, :].T).astype(BF16),
            "out_b_sl": out_b[vsl].reshape(1, VL),
            "ones1": ones1,
        })
    return in_maps


def _assemble(results):
    logp = np.concatenate([results[i]["logp_loc"] for i in range(NC)], axis=1)
    h_new = np.concatenate([results[i]["hnewT_loc"].T for i in range(NC)],
                           axis=1)[None]
    attn_w = np.concatenate([results[i]["attnw_loc"] for i in range(NC)],
                            axis=0)[:, None, :]
    return logp.astype(np.float32), h_new.astype(np.float32), \
        attn_w.astype(np.float32)


def kernel(**inputs):
    nc, _ = _build(repeat=1)
    in_maps = _prepare_in_maps(**inputs)
    res = run_bass_kernel_spmd(nc, in_maps, core_ids=list(range(NC)))
    return _assemble(res.results)
